# revision 1
# baseline (speedup 1.0000x reference)
"""AffinityPropagate Trainium2 kernel.

24 iterations of an 8-neighbor gated stencil:
    d <- (1-mask) * sum_k(gsh_k * shift_k(d)) / wsum + mask * blur

Strategy (8 NeuronCores, pure data parallel: one batch image per core):
  * Image [352, 1216] flattened row-major into SBUF [128 part x 3344].
  * Zero-padded 2D shifts == flat 1D shifted reads: per-direction gate
    weights are exactly 0 wherever a neighbor is out of bounds, so the
    row-wrap values the flat shift drags in are annihilated.
  * Per-direction weights W_k = |g_k shifted| * (1-mask)/wsum (fp16) are
    precomputed once; per iteration DVE does only the 8 fp16 multiplies
    (2x mode), PE sums the 8 product planes + b via identity-matmul PSUM
    accumulation, ACT casts PSUM->fp16 into ping-pong d tiles, and the
    +-1218-element halos are refreshed with SBUF->SBUF DMAs.
  * d_odd (d shifted by one element) keeps all shifted reads 4B-aligned
    so fp16 tensor_tensor stays in 2x mode for odd shift offsets.
"""

import numpy as np

from concourse import bass, mybir
from concourse.bass_utils import run_bass_kernel_spmd
from concourse.tile import TileContext

B, H, W = 8, 352, 1216
HW = H * W            # 428032
P = 128
F = HW // P           # 3344
HALO = 1218           # > max |shift| (1217), even
DW = HALO + F + HALO + 2  # d tile width (room for d_odd's +1 offset)
PROP_TIME = 24
NCHUNK_DVE = 4        # DVE multiply chunks per iteration
CD = F // NCHUNK_DVE  # 836
CP = 418              # one PSUM bank
OFFSETS = ((1, 1), (1, 0), (1, -1), (0, 1), (0, -1), (-1, 1), (-1, 0), (-1, -1))
SHIFTS = [dy * W + dx for dy, dx in OFFSETS]

f32 = mybir.dt.float32
f16 = mybir.dt.float16
MULT = mybir.AluOpType.mult
ADD = mybir.AluOpType.add

_CACHE = {}


def _split_sync_waits(nc, max_waits=1):
    """The walrus in this container accepts at most one sync-wait command
    per instruction; hoist extras onto preceding same-engine no-ops."""
    for f in nc.m.functions:
        for bb in f.blocks:
            out = []
            for inst in bb.instructions:
                si = inst.sync_info
                if si is not None and si.on_wait and len(si.on_wait) > max_waits:
                    waits = list(si.on_wait)
                    carry, keep = waits[:-max_waits], waits[-max_waits:]
                    for j, w in enumerate(carry):
                        out.append(mybir.InstNoOp(
                            name=f"{inst.name}-ws{j}", engine=inst.engine,
                            sync_info=mybir.SyncInfo(on_wait=[w], on_update=[]),
                            bass_nofuse=True))
                    inst.sync_info = mybir.SyncInfo(
                        on_wait=keep, on_update=list(si.on_update))
                out.append(inst)
            bb.instructions[:] = out


def _emit_shifted_plane_load(nc, gst, g, k, s, zrow):
    """gst[p, j] <- g[k, p*F + j + s], with every position whose 2D source
    is out of bounds forced to zero (rows here, wrap columns via masks)."""
    engs = (nc.sync, nc.scalar)
    eng = engs[k % 2]
    if s >= 0:
        for i, (p0, p1) in enumerate(((0, 32), (32, 64), (64, 96), (96, 127))):
            engs[(k + i) % 2].dma_start(
                out=gst[p0:p1, :],
                in_=g[k, s + p0 * F:s + p1 * F].rearrange(
                    "(p f) -> p f", p=p1 - p0))
        if s > 0:
            eng.dma_start(
                out=gst[127:128, 0:F - s],
                in_=g[k, s + 127 * F:HW].rearrange("(p f) -> p f", p=1))
        else:
            eng.dma_start(
                out=gst[127:128, :],
                in_=g[k, 127 * F:HW].rearrange("(p f) -> p f", p=1))
    else:
        a = -s
        eng.dma_start(
            out=gst[0:1, a:F],
            in_=g[k, 0:F - a].rearrange("(p f) -> p f", p=1))
        for i, (p0, p1) in enumerate(((1, 32), (32, 64), (64, 96), (96, 128))):
            engs[(k + i) % 2].dma_start(
                out=gst[p0:p1, :],
                in_=g[k, p0 * F - a:p1 * F - a].rearrange(
                    "(p f) -> p f", p=p1 - p0))
    dy = s // W if s >= 0 else -((-s + W - 1) // W)
    # top/bottom image rows (dy out of bounds) + DMA-uncovered slivers.
    # Partition-0 regions: DVE memset (legal start partition). Partition-127
    # regions: DMA from a zeros tile (compute APs must start on a quadrant).
    if s in (-1217, -1216, -1215):          # dy = -1
        nc.vector.memset(gst[0:1, 0:max(1216, -s)], 0.0)
    elif s == -1:
        nc.vector.memset(gst[0:1, 0:1], 0.0)
    elif s in (1215, 1216, 1217):           # dy = +1
        start = min(F - 1216, F - s)
        eng.dma_start(out=gst[127:128, start:F], in_=zrow[0:1, 0:F - start])
    elif s == 1:
        eng.dma_start(out=gst[127:128, F - 1:F], in_=zrow[0:1, 0:1])


def _halo_dmas(nc, d, dodd):
    """Refresh the flat-array halos of (d, d_odd) from d's own body."""
    # front halo of partition p = tail of partition p-1's body
    nc.sync.dma_start(out=d[1:128, 0:HALO], in_=d[0:127, F:F + HALO])
    nc.scalar.dma_start(out=dodd[1:128, 1:HALO + 1], in_=d[0:127, F:F + HALO])
    # back halo of partition p = head of partition p+1's body
    nc.sync.dma_start(out=d[0:127, HALO + F:HALO + F + HALO],
                      in_=d[1:128, HALO:2 * HALO])
    nc.scalar.dma_start(out=dodd[0:127, HALO + F + 1:HALO + F + HALO + 1],
                        in_=d[1:128, HALO:2 * HALO])


def _build():
    nc = bass.Bass()
    g = nc.dram_tensor("g", [8, HW], f32, kind="ExternalInput")
    blur = nc.dram_tensor("blur", [HW], f32, kind="ExternalInput")
    sparse = nc.dram_tensor("sparse", [HW], f32, kind="ExternalInput")
    maskL = nc.dram_tensor("maskL", [P, F], f16, kind="ExternalInput")
    maskR = nc.dram_tensor("maskR", [P, F], f16, kind="ExternalInput")
    ident = nc.dram_tensor("ident", [P, P], f16, kind="ExternalInput")
    out = nc.dram_tensor("out", [P, F], f32, kind="ExternalOutput")

    with TileContext(nc) as tc:
        with tc.tile_pool(name="const", bufs=1) as constp, \
             tc.tile_pool(name="wpool", bufs=1) as wpool, \
             tc.tile_pool(name="dpool", bufs=1) as dpool, \
             tc.tile_pool(name="misc", bufs=1) as miscp:

            identt = constp.tile([P, P], f16)
            nc.sync.dma_start(out=identt[:], in_=ident[:])
            zrow = constp.tile([P, 1220], f32)
            nc.vector.memset(zrow[:], 0.0)

            wt = [wpool.tile([P, F], f16, tag=f"w{k}", name=f"w{k}")
                  for k in range(8)]
            bt = miscp.tile([P, F], f16)

            dA = dpool.tile([P, DW], f16, tag="dA")
            dB = dpool.tile([P, DW], f16, tag="dB")
            doddA = dpool.tile([P, DW], f16, tag="doddA")
            doddB = dpool.tile([P, DW], f16, tag="doddB")
            for t in (dA, dB):
                nc.vector.memset(t[:, 0:HALO], 0.0)
                nc.vector.memset(t[:, HALO + F:DW], 0.0)
            for t in (doddA, doddB):
                # d_odd's body starts at HALO+1: cover j=HALO too (p0 keeps
                # reading it as the permanently-zero d_flat[-1])
                nc.vector.memset(t[:, 0:HALO + 1], 0.0)
                nc.vector.memset(t[:, HALO + F:DW], 0.0)

            # ---------------- preprocessing ----------------
            with tc.tile_pool(name="pre", bufs=2) as prep, \
                 tc.tile_pool(name="psumpre", bufs=4, space="PSUM") as psumpre:
                # guidance planes are the critical path: load them first
                maskLt = prep.tile([P, F], f16, tag="mL", bufs=1)
                maskRt = prep.tile([P, F], f16, tag="mR", bufs=1)
                nc.sync.dma_start(out=maskLt[:], in_=maskL[:])
                nc.scalar.dma_start(out=maskRt[:], in_=maskR[:])

                # wsum = sum of gate planes, accumulated on PE via identity
                # matmuls as each plane arrives (keeps DVE free)
                psw = [psumpre.tile([P, CP], f32, name=f"psw{q}", bufs=1,
                                    tag=f"psw{q}") for q in range(8)]
                # dx=0 planes last: their post-arrival chain skips the
                # wrap-column mask multiply
                for i, k in enumerate((0, 2, 3, 4, 5, 7, 1, 6)):
                    s = SHIFTS[k]
                    gst = prep.tile([P, F], f32, tag="gst", bufs=3)
                    _emit_shifted_plane_load(nc, gst, g, k, s, zrow)
                    # |g| -> fp16 gate plane
                    nc.scalar.activation(wt[k][:], gst[:],
                                         mybir.ActivationFunctionType.Abs)
                    dx = OFFSETS[k][1]
                    if dx == -1:
                        nc.vector.tensor_tensor(wt[k][:], wt[k][:], maskLt[:], MULT)
                    elif dx == 1:
                        nc.vector.tensor_tensor(wt[k][:], wt[k][:], maskRt[:], MULT)
                    for q in range(8):
                        qs = q * CP
                        nc.tensor.matmul(psw[q][:], identt[:],
                                         wt[k][:, qs:qs + CP],
                                         start=(i == 0), stop=(i == 7))

                # off the critical path: b / mask / d0
                sparse_st = prep.tile([P, F], f32, tag="gst", bufs=3)
                nc.sync.dma_start(
                    out=sparse_st[:], in_=sparse[:].rearrange("(p f) -> p f", p=P))
                blur_st = prep.tile([P, F], f32, tag="gst", bufs=3)
                nc.scalar.dma_start(
                    out=blur_st[:], in_=blur[:].rearrange("(p f) -> p f", p=P))
                m = prep.tile([P, F], f32, tag="m", bufs=1)
                nc.scalar.sign(m[:], sparse_st[:])
                nc.vector.tensor_tensor(bt[:], m[:], blur_st[:], MULT)  # b fp16
                # m <- 1 - m
                nc.vector.tensor_scalar(m[:], m[:], -1.0, 1.0, MULT, ADD)
                # d0 = blur (fp16 body + halos)
                nc.scalar.copy(out=dA[:, HALO:HALO + F], in_=blur_st[:])
                nc.scalar.copy(out=doddA[:, HALO + 1:HALO + 1 + F], in_=blur_st[:])
                _halo_dmas(nc, dA, doddA)

                # 1/wsum straight from the PSUM banks; chunked so iteration 1
                # can start on chunk 0 early
                winv = prep.tile([P, F], f32, tag="winv", bufs=1)
                winvh = prep.tile([P, F], f16, tag="mR", bufs=1)
                for c in range(NCHUNK_DVE):
                    sl = slice(c * CD, (c + 1) * CD)
                    for q in range(c * CD // CP, (c + 1) * CD // CP):
                        qs = q * CP
                        nc.vector.reciprocal(winv[:, qs:qs + CP], psw[q][:])
                    nc.vector.tensor_tensor(winv[:, sl], winv[:, sl],
                                            m[:, sl], MULT)
                    nc.scalar.copy(out=winvh[:, sl], in_=winv[:, sl])
                    # W_k = gate_k * (1-mask)/wsum   (fp16, in place, 2x mode)
                    for k in range(8):
                        nc.vector.tensor_tensor(wt[k][:, sl], wt[k][:, sl],
                                                winvh[:, sl], MULT)

            # ---------------- 24 stencil iterations ----------------
            with tc.tile_pool(name="prod", bufs=2) as prodp, \
                 tc.tile_pool(name="psum", bufs=4, space="PSUM") as psump, \
                 tc.tile_pool(name="post", bufs=1) as postp:

                src = (dA, doddA)
                dst = (dB, doddB)
                NSUB = CD // CP
                # chunks whose body feeds each halo side
                back_set = {c for c in range(NCHUNK_DVE) if c * CD < HALO}
                front_set = {c for c in range(NCHUNK_DVE)
                             if (c + 1) * CD > F - HALO}
                ostage = postp.tile([P, F], f32)
                for it in range(PROP_TIME):
                    last = it == PROP_TIME - 1
                    d_s, dodd_s = src
                    d_d, dodd_d = dst
                    order = range(NCHUNK_DVE) if it % 2 == 0 \
                        else range(NCHUNK_DVE - 1, -1, -1)
                    back_done = front_done = False
                    done = set()
                    for c in order:
                        cs = c * CD
                        prods = []
                        for k, s in enumerate(SHIFTS):
                            if s % 2 == 0:
                                base = HALO + s
                                srct = d_s
                            else:
                                base = HALO + 1 + s
                                srct = dodd_s
                            pr = prodp.tile([P, CD], f16, tag=f"pr{k}")
                            nc.vector.tensor_tensor(
                                pr[:], wt[k][:, cs:cs + CD],
                                srct[:, base + cs:base + cs + CD], MULT)
                            prods.append(pr)
                        for h in range(NSUB):
                            hs = h * CP
                            ps = psump.tile([P, CP], f32)
                            nc.tensor.matmul(ps[:], identt[:],
                                             bt[:, cs + hs:cs + hs + CP],
                                             start=True, stop=False)
                            for k in range(8):
                                nc.tensor.matmul(ps[:], identt[:],
                                                 prods[k][:, hs:hs + CP],
                                                 start=False, stop=(k == 7))
                            if last:
                                # stream final result straight to DRAM (fp32)
                                nc.scalar.copy(
                                    out=ostage[:, cs + hs:cs + hs + CP],
                                    in_=ps[:])
                                nc.sync.dma_start(
                                    out=out[:, cs + hs:cs + hs + CP],
                                    in_=ostage[:, cs + hs:cs + hs + CP])
                            else:
                                nc.scalar.copy(
                                    out=d_d[:, HALO + cs + hs:HALO + cs + hs + CP],
                                    in_=ps[:])
                                nc.scalar.copy(
                                    out=dodd_d[:, HALO + 1 + cs + hs:HALO + 1 + cs + hs + CP],
                                    in_=ps[:])
                        if last:
                            continue
                        done.add(c)
                        # launch halo refreshes as soon as their source body
                        # chunks have been written
                        if not back_done and back_set <= done:
                            nc.sync.dma_start(
                                out=d_d[0:127, HALO + F:HALO + F + HALO],
                                in_=d_d[1:128, HALO:2 * HALO])
                            nc.scalar.dma_start(
                                out=dodd_d[0:127, HALO + F + 1:HALO + F + HALO + 1],
                                in_=d_d[1:128, HALO:2 * HALO])
                            back_done = True
                        if not front_done and front_set <= done:
                            nc.sync.dma_start(
                                out=d_d[1:128, 0:HALO],
                                in_=d_d[0:127, F:F + HALO])
                            nc.scalar.dma_start(
                                out=dodd_d[1:128, 1:HALO + 1],
                                in_=d_d[0:127, F:F + HALO])
                            front_done = True
                    src, dst = dst, src

    nc.finalize()
    _split_sync_waits(nc)
    return nc


def _consts():
    j = np.arange(HW, dtype=np.int64) % W
    mL = (j != 0).astype(np.float16).reshape(P, F)
    mR = (j != W - 1).astype(np.float16).reshape(P, F)
    return mL, mR, np.eye(P, dtype=np.float16)


def kernel(guidance, blur_depth, sparse_depth):
    if "nc" not in _CACHE:
        _CACHE["nc"] = _build()
    nc = _CACHE["nc"]
    guidance = np.asarray(guidance, dtype=np.float32)
    blur_depth = np.asarray(blur_depth, dtype=np.float32)
    sparse_depth = np.asarray(sparse_depth, dtype=np.float32)
    mL, mR, idm = _consts()
    in_maps = []
    for c in range(B):
        in_maps.append({
            "g": np.ascontiguousarray(guidance[c].reshape(8, HW)),
            "blur": np.ascontiguousarray(blur_depth[c].reshape(HW)),
            "sparse": np.ascontiguousarray(sparse_depth[c].reshape(HW)),
            "maskL": mL, "maskR": mR, "ident": idm,
        })
    # every iterate is a convex combination of blur_depth values, so the
    # output must stay inside blur's range; violations mean the device
    # glitched (transient NRT wedge) -> retry
    lo = float(blur_depth.min()) - 1e-2
    hi = float(blur_depth.max()) + 1e-2

    import time
    outp = None
    for attempt in range(4):
        try:
            res = run_bass_kernel_spmd(nc, in_maps, list(range(B)))
            outp = np.stack(
                [res.results[c]["out"].reshape(1, H, W) for c in range(B)])
            if np.isfinite(outp).all() and outp.min() >= lo and outp.max() <= hi:
                return outp
            print(f"kernel: attempt {attempt} produced out-of-range values; "
                  f"retrying", flush=True)
        except Exception as e:
            # transient NRT device-unrecoverable states clear on a retry
            if attempt == 3:
                raise
            print(f"kernel: attempt {attempt} failed ({type(e).__name__}); "
                  f"retrying", flush=True)
        time.sleep(20 * (attempt + 1))
    return outp



# revision 13
# speedup vs baseline: 1.1608x; 1.1608x over previous
"""AffinityPropagate Trainium2 kernel.

24 iterations of an 8-neighbor gated stencil:
    d <- (1-mask) * sum_k(gsh_k * shift_k(d)) / wsum + mask * blur

Strategy (8 NeuronCores, pure data parallel: one batch image per core):
  * Image [352, 1216] flattened row-major into SBUF [128 part x 3344].
  * Zero-padded 2D shifts == flat 1D shifted reads: per-direction gate
    weights are exactly 0 wherever a neighbor is out of bounds, so the
    row-wrap values the flat shift drags in are annihilated.
  * Per-direction weights W_k = |g_k shifted| * (1-mask)/wsum (fp16) are
    precomputed once.  Per iteration the 32 product chunks (8 dirs x 4
    chunks) are split DVE:Pool = 25:7 so the two elementwise engines
    finish together; PE sums the 8 product planes + b via identity-matmul
    PSUM accumulation (9 streams); ACT casts PSUM->fp16 into ping-pong d
    tiles; the +-1218-element halos are refreshed with SBUF->SBUF DMAs.
  * Chunk order alternates per iteration so the last-written chunk of
    iteration i is the first one consumed by iteration i+1.
"""

import numpy as np

from concourse import bass, mybir
from concourse.bass_utils import run_bass_kernel_spmd
from concourse.tile import TileContext

B, H, W = 8, 352, 1216
HW = H * W            # 428032
P = 128
F = HW // P           # 3344
HALO = 1218           # > max |shift| (1217), even
DW = HALO + F + HALO  # d tile width
PROP_TIME = 24
CP = 418              # one PSUM bank
# product chunks; the ends stay small-ish so the iteration boundary (last
# chunk written -> first chunk read next iteration) drains through PE/ACT
# quickly
CHUNKS = ((0, 836), (836, 836), (1672, 836), (2508, 836))
NCHUNK = len(CHUNKS)
OFFSETS = ((1, 1), (1, 0), (1, -1), (0, 1), (0, -1), (-1, 1), (-1, 0), (-1, -1))
SHIFTS = [dy * W + dx for dy, dx in OFFSETS]
# product chunks done on the (slower) Pool/GPSIMD engine instead of DVE;
# sized so DVE (planes 0-5 + plane 6 edge) and Pool finish together
POOL_TASKS = {(7, 0), (7, 1), (7, 2), (7, 3), (6, 1), (6, 2), (6, 3)}

f32 = mybir.dt.float32
f16 = mybir.dt.float16
MULT = mybir.AluOpType.mult
ADD = mybir.AluOpType.add

_CACHE = {}


def _split_sync_waits(nc, max_waits=1):
    """The walrus in this container accepts at most one sync-wait command
    per instruction; hoist extras onto preceding same-engine no-ops."""
    for f in nc.m.functions:
        for bb in f.blocks:
            out = []
            for inst in bb.instructions:
                si = inst.sync_info
                if si is not None and si.on_wait and len(si.on_wait) > max_waits:
                    waits = list(si.on_wait)
                    carry, keep = waits[:-max_waits], waits[-max_waits:]
                    for j, w in enumerate(carry):
                        out.append(mybir.InstNoOp(
                            name=f"{inst.name}-ws{j}", engine=inst.engine,
                            sync_info=mybir.SyncInfo(on_wait=[w], on_update=[]),
                            bass_nofuse=True))
                    inst.sync_info = mybir.SyncInfo(
                        on_wait=keep, on_update=list(si.on_update))
                out.append(inst)
            bb.instructions[:] = out


def _emit_shifted_plane_load(nc, gst, g, k, s, zrow):
    """gst[p, j] <- g[k, p*F + j + s], with every position whose 2D source
    is out of bounds forced to zero (rows here, wrap columns via masks)."""
    engs = (nc.sync, nc.scalar)
    eng = engs[k % 2]
    if s >= 0:
        for i, (p0, p1) in enumerate(((0, 32), (32, 64), (64, 96), (96, 127))):
            engs[(k + i) % 2].dma_start(
                out=gst[p0:p1, :],
                in_=g[k, s + p0 * F:s + p1 * F].rearrange(
                    "(p f) -> p f", p=p1 - p0))
        if s > 0:
            eng.dma_start(
                out=gst[127:128, 0:F - s],
                in_=g[k, s + 127 * F:HW].rearrange("(p f) -> p f", p=1))
        else:
            eng.dma_start(
                out=gst[127:128, :],
                in_=g[k, 127 * F:HW].rearrange("(p f) -> p f", p=1))
    else:
        a = -s
        eng.dma_start(
            out=gst[0:1, a:F],
            in_=g[k, 0:F - a].rearrange("(p f) -> p f", p=1))
        for i, (p0, p1) in enumerate(((1, 32), (32, 64), (64, 96), (96, 128))):
            engs[(k + i) % 2].dma_start(
                out=gst[p0:p1, :],
                in_=g[k, p0 * F - a:p1 * F - a].rearrange(
                    "(p f) -> p f", p=p1 - p0))
    dy = s // W if s >= 0 else -((-s + W - 1) // W)
    # top/bottom image rows (dy out of bounds) + DMA-uncovered slivers.
    # Partition-0 regions: DVE memset (legal start partition). Partition-127
    # regions: DMA from a zeros tile (compute APs must start on a quadrant).
    if s in (-1217, -1216, -1215):          # dy = -1
        nc.vector.memset(gst[0:1, 0:max(1216, -s)], 0.0)
    elif s == -1:
        nc.vector.memset(gst[0:1, 0:1], 0.0)
    elif s in (1215, 1216, 1217):           # dy = +1
        start = min(F - 1216, F - s)
        eng.dma_start(out=gst[127:128, start:F], in_=zrow[0:1, 0:F - start])
    elif s == 1:
        eng.dma_start(out=gst[127:128, F - 1:F], in_=zrow[0:1, 0:1])


def _halo_dmas(nc, d):
    """Refresh the flat-array halos of d from d's own body."""
    # front halo of partition p = tail of partition p-1's body
    nc.sync.dma_start(out=d[1:128, 0:HALO], in_=d[0:127, F:F + HALO])
    # back halo of partition p = head of partition p+1's body
    nc.scalar.dma_start(out=d[0:127, HALO + F:HALO + F + HALO],
                        in_=d[1:128, HALO:2 * HALO])


def _build():
    nc = bass.Bass()
    g = nc.dram_tensor("g", [8, HW], f32, kind="ExternalInput")
    blur = nc.dram_tensor("blur", [HW], f32, kind="ExternalInput")
    sparse = nc.dram_tensor("sparse", [HW], f32, kind="ExternalInput")
    maskL = nc.dram_tensor("maskL", [P, F], f16, kind="ExternalInput")
    maskR = nc.dram_tensor("maskR", [P, F], f16, kind="ExternalInput")
    ident = nc.dram_tensor("ident", [P, P], f16, kind="ExternalInput")
    out = nc.dram_tensor("out", [P, F], f32, kind="ExternalOutput")

    with TileContext(nc) as tc:
        with tc.tile_pool(name="const", bufs=1) as constp, \
             tc.tile_pool(name="wpool", bufs=1) as wpool, \
             tc.tile_pool(name="dpool", bufs=1) as dpool, \
             tc.tile_pool(name="misc", bufs=1) as miscp:

            identt = constp.tile([P, P], f16)
            nc.sync.dma_start(out=identt[:], in_=ident[:])
            zrow = constp.tile([P, 1220], f32)
            nc.gpsimd.memset(zrow[:], 0.0)

            wt = [wpool.tile([P, F], f16, tag=f"w{k}", name=f"w{k}")
                  for k in range(8)]
            bt = miscp.tile([P, F], f16)

            dA = dpool.tile([P, DW], f16, tag="dA")
            dB = dpool.tile([P, DW], f16, tag="dB")
            for t in (dA, dB):
                nc.gpsimd.memset(t[:, 0:HALO], 0.0)
                nc.gpsimd.memset(t[:, HALO + F:DW], 0.0)

            # ---------------- preprocessing ----------------
            # the DMA queue (one shared resource) is the pre bottleneck:
            # sparse first (mask chain), guidance planes back to back, blur
            # mid-window, and the two dx=0 planes last, column-chunked, so
            # wsum bank 0 completes right after the last chunk lands.  ACT
            # runs only sign + the abs chain; everything else is DVE/Pool.
            prew = wpool.tile([P, F], f16, tag="winvh", name="prew")
            with tc.tile_pool(name="pre", bufs=2) as prep, \
                 tc.tile_pool(name="psumpre", bufs=4, space="PSUM") as psumpre:
                maskLt = prep.tile([P, F], f16, tag="mL", bufs=1)
                maskRt = prep.tile([P, F], f16, tag="mR", bufs=1)
                nc.sync.dma_start(out=maskLt[:], in_=maskL[:])
                # maskR[j] == maskL[j+1]: derive on-chip, fetch only the
                # last column from DRAM
                nc.scalar.dma_start(out=maskRt[:, F - 1:F],
                                    in_=maskR[:, F - 1:F])
                nc.vector.tensor_copy(out=maskRt[:, 0:F - 1],
                                      in_=maskLt[:, 1:F])
                sparse_st = prep.tile([P, F], f32, tag="gst", bufs=3)
                nc.sync.dma_start(
                    out=sparse_st[:], in_=sparse[:].rearrange("(p f) -> p f", p=P))
                m = prep.tile([P, F], f32, tag="m", bufs=1)
                mh = prep.tile([P, F], f16, tag="mh", bufs=1)
                nc.scalar.sign(m[:], sparse_st[:])
                nc.vector.tensor_copy(out=mh[:], in_=m[:])
                # mh <- 1 - mask (fp16, 4x mode)
                nc.vector.tensor_scalar(mh[:], mh[:], -1.0, 1.0, MULT, ADD)

                # wsum = sum of gate planes, accumulated on PE via identity
                # matmuls as each plane arrives (keeps DVE free)
                psw = [psumpre.tile([P, CP], f32, name=f"psw{q}", bufs=1,
                                    tag=f"psw{q}") for q in range(8)]
                blur_st = None
                for i, k in enumerate((0, 2, 3, 4, 5, 7)):
                    s = SHIFTS[k]
                    gst = prep.tile([P, F], f32, tag="gst", bufs=3)
                    _emit_shifted_plane_load(nc, gst, g, k, s, zrow)
                    # |g| -> fp16 gate plane
                    nc.scalar.activation(wt[k][:], gst[:],
                                         mybir.ActivationFunctionType.Abs)
                    # wrap-column zeroing: Pool for the early planes (it
                    # idles all window), DVE for the last one
                    mt = maskLt if OFFSETS[k][1] == -1 else maskRt
                    eng = nc.vector if k == 7 else nc.gpsimd
                    eng.tensor_tensor(wt[k][:], wt[k][:], mt[:], MULT)
                    for q in range(8):
                        qs = q * CP
                        nc.tensor.matmul(psw[q][:], identt[:],
                                         wt[k][:, qs:qs + CP],
                                         start=(i == 0), stop=False)
                    if i == 3:
                        # blur mid-window: d0 / b ready long before the
                        # last gate plane lands
                        blur_st = prep.tile([P, F], f32, tag="gst", bufs=3)
                        nc.scalar.dma_start(
                            out=blur_st[:],
                            in_=blur[:].rearrange("(p f) -> p f", p=P))
                        # d0 = blur (fp16 body + halos); DVE so the ACT abs
                        # chain is never delayed
                        nc.vector.tensor_copy(out=dA[:, HALO:HALO + F],
                                              in_=blur_st[:])
                        _halo_dmas(nc, dA)
                        nc.vector.tensor_tensor(bt[:], m[:], blur_st[:], MULT)

                # dx=0 planes (no mask multiply) last, loaded column-chunked
                # and abs'd chunk-by-chunk: wsum bank 0 closes right after
                # the first chunks land instead of after the full planes
                gst1 = prep.tile([P, F], f32, tag="gst1", bufs=1)
                gst6 = prep.tile([P, F], f32, tag="gst6", bufs=1)
                s1, s6 = SHIFTS[1], SHIFTS[6]      # +1216 / -1216
                a6 = -s6
                nc.vector.memset(gst6[0:1, 0:a6], 0.0)
                lim1 = F - s1  # partition-127 in-bounds columns for k1
                for ci, (cs, cw) in enumerate(CHUNKS):
                    # k1 (s=+1216): partitions 0..126 shifted, 127 special
                    nc.sync.dma_start(
                        out=gst1[0:127, cs:cs + cw],
                        in_=g[1, s1:s1 + 127 * F].rearrange(
                            "(p f) -> p f", p=127)[:, cs:cs + cw])
                    if cs < lim1:
                        w = min(cs + cw, lim1) - cs
                        nc.sync.dma_start(
                            out=gst1[127:128, cs:cs + w],
                            in_=g[1, s1 + 127 * F + cs:s1 + 127 * F + cs + w]
                            .rearrange("(p f) -> p f", p=1))
                    if cs + cw > lim1:
                        zs = max(cs, lim1)
                        nc.sync.dma_start(out=gst1[127:128, zs:cs + cw],
                                          in_=zrow[0:1, 0:cs + cw - zs])
                    # k6 (s=-1216): partitions 1..127 shifted, 0 special
                    nc.scalar.dma_start(
                        out=gst6[1:128, cs:cs + cw],
                        in_=g[6, F - a6:F - a6 + 127 * F].rearrange(
                            "(p f) -> p f", p=127)[:, cs:cs + cw])
                    if cs + cw > a6:
                        zs = max(cs, a6)
                        nc.scalar.dma_start(
                            out=gst6[0:1, zs:cs + cw],
                            in_=g[6, zs - a6:cs + cw - a6].rearrange(
                                "(p f) -> p f", p=1))
                    for kk, gstk in ((1, gst1), (6, gst6)):
                        nc.scalar.activation(
                            wt[kk][:, cs:cs + cw], gstk[:, cs:cs + cw],
                            mybir.ActivationFunctionType.Abs)
                        for q in range(cs // CP, (cs + cw) // CP):
                            qs = q * CP
                            nc.tensor.matmul(psw[q][:], identt[:],
                                             wt[kk][:, qs:qs + CP],
                                             start=False, stop=(kk == 6))

                # winv = (1-mask)/wsum per chunk; the W_k normalize runs
                # fused into iteration 0 below
                winvh = prew
                for cs, cw in CHUNKS:
                    sl = slice(cs, cs + cw)
                    winv = prep.tile([P, CP], f32, tag="winv", bufs=2)
                    for q in range(cs // CP, (cs + cw) // CP):
                        qs = q * CP
                        nc.vector.reciprocal(winv[:], psw[q][:])
                        nc.vector.tensor_copy(out=winvh[:, qs:qs + CP],
                                              in_=winv[:])
                    nc.vector.tensor_tensor(winvh[:, sl], winvh[:, sl],
                                            mh[:, sl], MULT)

            # ---------------- 24 stencil iterations ----------------
            with tc.tile_pool(name="prod", bufs=3) as prodp, \
                 tc.tile_pool(name="psum", bufs=8, space="PSUM") as psump, \
                 tc.tile_pool(name="post", bufs=1) as postp:

                src, dst = dA, dB
                # chunks whose body feeds each halo side
                back_set = {c for c, (cs, cw) in enumerate(CHUNKS)
                            if cs < HALO}
                front_set = {c for c, (cs, cw) in enumerate(CHUNKS)
                             if cs + cw > F - HALO}
                ostage = postp.tile([P, F], f32)
                WMAX = max(cw for cs, cw in CHUNKS)
                for it in range(PROP_TIME):
                    last = it == PROP_TIME - 1
                    order = range(NCHUNK) if it % 2 == 0 \
                        else range(NCHUNK - 1, -1, -1)
                    back_done = front_done = False
                    done = set()
                    # prods[k] = (tile, base_col): planes 0-5 compute the two
                    # middle chunks as one 1672-wide DVE op (fewer per-inst
                    # setup overheads); boundary chunks stay narrow
                    prods = [None] * 8
                    for c in order:
                        cs, cw = CHUNKS[c]
                        if it == 0:
                            # W_k = |g_k| * (1-mask)/wsum, fused chunk-wise
                            # into the first iteration so DVE never sits on
                            # a full normalize pass before iterating
                            for k in range(8):
                                eng = nc.gpsimd if k >= 6 else nc.vector
                                eng.tensor_tensor(
                                    wt[k][:, cs:cs + cw], wt[k][:, cs:cs + cw],
                                    prew[:, cs:cs + cw], MULT)
                        # vertical-shift planes (|s| >= 1215) read far from
                        # the chunk just written by the previous iteration;
                        # issue them first so the horizontal planes (s = +-1,
                        # which wait on the freshest PSUM->SBUF copies) don't
                        # stall the engine at iteration boundaries
                        dve_ks = [k for k in (0, 1, 2, 5, 6, 3, 4)
                                  if (k, c) not in POOL_TASKS]
                        pool_ks = [k for k in (6, 7) if (k, c) in POOL_TASKS]
                        for k in dve_ks + pool_ks:
                            ps_, pw = cs, cw
                            if k < 6 and c in (1, 2):
                                if prods[k] is not None \
                                        and prods[k][1] == CHUNKS[1][0]:
                                    continue  # other middle half already done
                                ps_, pw = CHUNKS[1][0], \
                                    CHUNKS[2][0] + CHUNKS[2][1] - CHUNKS[1][0]
                            s = SHIFTS[k]
                            base = HALO + s
                            pr = prodp.tile([P, WMAX], f16, tag=f"pr{k}",
                                            name=f"pr{k}")
                            eng = nc.gpsimd if (k, c) in POOL_TASKS \
                                else nc.vector
                            eng.tensor_tensor(
                                pr[:, 0:pw], wt[k][:, ps_:ps_ + pw],
                                src[:, base + ps_:base + ps_ + pw], MULT)
                            prods[k] = (pr, ps_)
                        for h in range(cw // CP):
                            hs = cs + h * CP
                            ps = psump.tile([P, CP], f32, name="ps")
                            nc.tensor.matmul(ps[:], identt[:],
                                             bt[:, hs:hs + CP],
                                             start=True, stop=False)
                            for k in range(8):
                                pk, pb = prods[k]
                                nc.tensor.matmul(
                                    ps[:], identt[:],
                                    pk[:, hs - pb:hs - pb + CP],
                                    start=False, stop=(k == 7))
                            if last:
                                # stream final result straight to DRAM (fp32)
                                nc.scalar.copy(
                                    out=ostage[:, cs + hs:cs + hs + CP],
                                    in_=ps[:])
                                nc.sync.dma_start(
                                    out=out[:, cs + hs:cs + hs + CP],
                                    in_=ostage[:, cs + hs:cs + hs + CP])
                            else:
                                nc.scalar.copy(
                                    out=dst[:, HALO + cs + hs:HALO + cs + hs + CP],
                                    in_=ps[:])
                        if last:
                            continue
                        done.add(c)
                        # launch halo refreshes as soon as their source body
                        # chunks have been written
                        if not back_done and back_set <= done:
                            nc.sync.dma_start(
                                out=dst[0:127, HALO + F:HALO + F + HALO],
                                in_=dst[1:128, HALO:2 * HALO])
                            back_done = True
                        if not front_done and front_set <= done:
                            nc.scalar.dma_start(
                                out=dst[1:128, 0:HALO],
                                in_=dst[0:127, F:F + HALO])
                            front_done = True
                    src, dst = dst, src

    nc.finalize()
    _split_sync_waits(nc)
    return nc


def _consts():
    j = np.arange(HW, dtype=np.int64) % W
    mL = (j != 0).astype(np.float16).reshape(P, F)
    mR = (j != W - 1).astype(np.float16).reshape(P, F)
    return mL, mR, np.eye(P, dtype=np.float16)


def kernel(guidance, blur_depth, sparse_depth):
    if "nc" not in _CACHE:
        _CACHE["nc"] = _build()
    nc = _CACHE["nc"]
    guidance = np.asarray(guidance, dtype=np.float32)
    blur_depth = np.asarray(blur_depth, dtype=np.float32)
    sparse_depth = np.asarray(sparse_depth, dtype=np.float32)
    mL, mR, idm = _consts()
    in_maps = []
    for c in range(B):
        in_maps.append({
            "g": np.ascontiguousarray(guidance[c].reshape(8, HW)),
            "blur": np.ascontiguousarray(blur_depth[c].reshape(HW)),
            "sparse": np.ascontiguousarray(sparse_depth[c].reshape(HW)),
            "maskL": mL, "maskR": mR, "ident": idm,
        })
    # every iterate is a convex combination of blur_depth values, so the
    # output must stay inside blur's range; violations mean the device
    # glitched (transient NRT wedge) -> retry
    lo = float(blur_depth.min()) - 1e-2
    hi = float(blur_depth.max()) + 1e-2

    import time
    outp = None
    for attempt in range(4):
        try:
            res = run_bass_kernel_spmd(nc, in_maps, list(range(B)))
            outp = np.stack(
                [res.results[c]["out"].reshape(1, H, W) for c in range(B)])
            if np.isfinite(outp).all() and outp.min() >= lo and outp.max() <= hi:
                return outp
            print(f"kernel: attempt {attempt} produced out-of-range values; "
                  f"retrying", flush=True)
        except Exception as e:
            # transient NRT device-unrecoverable states clear on a retry
            if attempt == 3:
                raise
            print(f"kernel: attempt {attempt} failed ({type(e).__name__}); "
                  f"retrying", flush=True)
        time.sleep(20 * (attempt + 1))
    return outp


# revision 25
# speedup vs baseline: 1.1693x; 1.0073x over previous
"""AffinityPropagate Trainium2 kernel.

24 iterations of an 8-neighbor gated stencil:
    d <- (1-mask) * sum_k(gsh_k * shift_k(d)) / wsum + mask * blur

Strategy (8 NeuronCores, pure data parallel: one batch image per core):
  * Image [352, 1216] flattened row-major into SBUF [128 part x 3344].
  * Zero-padded 2D shifts == flat 1D shifted reads: per-direction gate
    weights are exactly 0 wherever a neighbor is out of bounds, so the
    row-wrap values the flat shift drags in are annihilated.
  * Per-direction weights W_k = |g_k shifted| * (1-mask)/wsum (fp16) are
    precomputed once.  Per iteration the 32 product chunks (8 dirs x 4
    chunks) are split DVE:Pool = 25:7 so the two elementwise engines
    finish together; PE sums the 8 product planes + b via identity-matmul
    PSUM accumulation (9 streams); ACT casts PSUM->fp16 into ping-pong d
    tiles; the +-1218-element halos are refreshed with SBUF->SBUF DMAs.
  * Chunk order alternates per iteration so the last-written chunk of
    iteration i is the first one consumed by iteration i+1.
"""

import numpy as np

from concourse import bass, mybir
from concourse.bass_utils import run_bass_kernel_spmd
from concourse.tile import TileContext

B, H, W = 8, 352, 1216
HW = H * W            # 428032
P = 128
F = HW // P           # 3344
HALO = 1218           # > max |shift| (1217), even
DW = HALO + F + HALO  # d tile width
PROP_TIME = 24
CP = 418              # one PSUM bank
# product chunks; the ends stay small-ish so the iteration boundary (last
# chunk written -> first chunk read next iteration) drains through PE/ACT
# quickly
CHUNKS = ((0, 836), (836, 836), (1672, 836), (2508, 836))
NCHUNK = len(CHUNKS)
OFFSETS = ((1, 1), (1, 0), (1, -1), (0, 1), (0, -1), (-1, 1), (-1, 0), (-1, -1))
SHIFTS = [dy * W + dx for dy, dx in OFFSETS]
# product chunks done on the (slower) Pool/GPSIMD engine instead of DVE;
# sized so DVE (planes 0-5 + plane 6 edge) and Pool finish together
POOL_TASKS = {(7, 0), (7, 1), (7, 2), (7, 3), (6, 1), (6, 2), (6, 3)}

f32 = mybir.dt.float32
f16 = mybir.dt.float16
MULT = mybir.AluOpType.mult
ADD = mybir.AluOpType.add

_CACHE = {}


def _split_sync_waits(nc, max_waits=1):
    """The walrus in this container accepts at most one sync-wait command
    per instruction; hoist extras onto preceding same-engine no-ops."""
    for f in nc.m.functions:
        for bb in f.blocks:
            out = []
            for inst in bb.instructions:
                si = inst.sync_info
                if si is not None and si.on_wait and len(si.on_wait) > max_waits:
                    waits = list(si.on_wait)
                    carry, keep = waits[:-max_waits], waits[-max_waits:]
                    for j, w in enumerate(carry):
                        out.append(mybir.InstNoOp(
                            name=f"{inst.name}-ws{j}", engine=inst.engine,
                            sync_info=mybir.SyncInfo(on_wait=[w], on_update=[]),
                            bass_nofuse=True))
                    inst.sync_info = mybir.SyncInfo(
                        on_wait=keep, on_update=list(si.on_update))
                out.append(inst)
            bb.instructions[:] = out


def _emit_shifted_plane_load(nc, gst, g, k, s, zrow):
    """gst[p, j] <- g[k, p*F + j + s], with every position whose 2D source
    is out of bounds forced to zero (rows here, wrap columns via masks)."""
    engs = (nc.sync, nc.scalar)
    eng = engs[k % 2]
    if s >= 0:
        for i, (p0, p1) in enumerate(((0, 32), (32, 64), (64, 96), (96, 127))):
            engs[(k + i) % 2].dma_start(
                out=gst[p0:p1, :],
                in_=g[k, s + p0 * F:s + p1 * F].rearrange(
                    "(p f) -> p f", p=p1 - p0))
        if s > 0:
            eng.dma_start(
                out=gst[127:128, 0:F - s],
                in_=g[k, s + 127 * F:HW].rearrange("(p f) -> p f", p=1))
        else:
            eng.dma_start(
                out=gst[127:128, :],
                in_=g[k, 127 * F:HW].rearrange("(p f) -> p f", p=1))
    else:
        a = -s
        eng.dma_start(
            out=gst[0:1, a:F],
            in_=g[k, 0:F - a].rearrange("(p f) -> p f", p=1))
        for i, (p0, p1) in enumerate(((1, 32), (32, 64), (64, 96), (96, 128))):
            engs[(k + i) % 2].dma_start(
                out=gst[p0:p1, :],
                in_=g[k, p0 * F - a:p1 * F - a].rearrange(
                    "(p f) -> p f", p=p1 - p0))
    dy = s // W if s >= 0 else -((-s + W - 1) // W)
    # top/bottom image rows (dy out of bounds) + DMA-uncovered slivers.
    # Partition-0 regions: DVE memset (legal start partition). Partition-127
    # regions: DMA from a zeros tile (compute APs must start on a quadrant).
    if s in (-1217, -1216, -1215):          # dy = -1
        nc.vector.memset(gst[0:1, 0:max(1216, -s)], 0.0)
    elif s == -1:
        nc.vector.memset(gst[0:1, 0:1], 0.0)
    elif s in (1215, 1216, 1217):           # dy = +1
        start = min(F - 1216, F - s)
        eng.dma_start(out=gst[127:128, start:F], in_=zrow[0:1, 0:F - start])
    elif s == 1:
        eng.dma_start(out=gst[127:128, F - 1:F], in_=zrow[0:1, 0:1])


def _halo_dmas(nc, d):
    """Refresh the flat-array halos of d from d's own body."""
    # front halo of partition p = tail of partition p-1's body
    nc.sync.dma_start(out=d[1:128, 0:HALO], in_=d[0:127, F:F + HALO])
    # back halo of partition p = head of partition p+1's body
    nc.scalar.dma_start(out=d[0:127, HALO + F:HALO + F + HALO],
                        in_=d[1:128, HALO:2 * HALO])


def _build():
    nc = bass.Bass()
    g = nc.dram_tensor("g", [8, HW], f32, kind="ExternalInput")
    blur = nc.dram_tensor("blur", [HW], f32, kind="ExternalInput")
    sparse = nc.dram_tensor("sparse", [HW], f32, kind="ExternalInput")
    maskL = nc.dram_tensor("maskL", [P, F], f16, kind="ExternalInput")
    maskR = nc.dram_tensor("maskR", [P, F], f16, kind="ExternalInput")
    ident = nc.dram_tensor("ident", [P, P], f16, kind="ExternalInput")
    out = nc.dram_tensor("out", [P, F], f32, kind="ExternalOutput")

    with TileContext(nc) as tc:
        with tc.tile_pool(name="const", bufs=1) as constp, \
             tc.tile_pool(name="wpool", bufs=1) as wpool, \
             tc.tile_pool(name="dpool", bufs=1) as dpool, \
             tc.tile_pool(name="misc", bufs=1) as miscp:

            identt = constp.tile([P, P], f16)
            nc.sync.dma_start(out=identt[:], in_=ident[:])
            zrow = constp.tile([P, 1220], f32)
            nc.gpsimd.memset(zrow[:], 0.0)

            wt = [wpool.tile([P, F], f16, tag=f"w{k}", name=f"w{k}")
                  for k in range(8)]
            bt = miscp.tile([P, F], f16)

            dA = dpool.tile([P, DW], f16, tag="dA")
            dB = dpool.tile([P, DW], f16, tag="dB")
            for t in (dA, dB):
                nc.gpsimd.memset(t[:, 0:HALO], 0.0)
                nc.gpsimd.memset(t[:, HALO + F:DW], 0.0)

            # ---------------- preprocessing ----------------
            # the DMA queue (one shared resource) is the pre bottleneck:
            # sparse first (mask chain), guidance planes back to back, blur
            # mid-window, and the two dx=0 planes last, column-chunked, so
            # wsum bank 0 completes right after the last chunk lands.  ACT
            # runs only sign + the abs chain; everything else is DVE/Pool.
            prew = wpool.tile([P, F], f16, tag="winvh", name="prew")
            with tc.tile_pool(name="pre", bufs=2) as prep, \
                 tc.tile_pool(name="psumpre", bufs=4, space="PSUM") as psumpre:
                maskLt = prep.tile([P, F], f16, tag="mL", bufs=1)
                maskRt = prep.tile([P, F], f16, tag="mR", bufs=1)
                nc.sync.dma_start(out=maskLt[:], in_=maskL[:])
                # maskR[j] == maskL[j+1]: derive on-chip, fetch only the
                # last column from DRAM
                nc.scalar.dma_start(out=maskRt[:, F - 1:F],
                                    in_=maskR[:, F - 1:F])
                nc.vector.tensor_copy(out=maskRt[:, 0:F - 1],
                                      in_=maskLt[:, 1:F])
                sparse_st = prep.tile([P, F], f32, tag="gst", bufs=3)
                nc.sync.dma_start(
                    out=sparse_st[:], in_=sparse[:].rearrange("(p f) -> p f", p=P))
                m = prep.tile([P, F], f32, tag="m", bufs=1)
                mh = prep.tile([P, F], f16, tag="mh", bufs=1)
                nc.scalar.sign(m[:], sparse_st[:])
                nc.vector.tensor_copy(out=mh[:], in_=m[:])
                # mh <- 1 - mask (fp16, 4x mode)
                nc.vector.tensor_scalar(mh[:], mh[:], -1.0, 1.0, MULT, ADD)

                # wsum = sum of gate planes, accumulated on PE via identity
                # matmuls as each plane arrives (keeps DVE free)
                psw = [psumpre.tile([P, CP], f32, name=f"psw{q}", bufs=1,
                                    tag=f"psw{q}") for q in range(8)]
                blur_st = None
                for i, k in enumerate((0, 2, 3, 4, 5, 7)):
                    s = SHIFTS[k]
                    gst = prep.tile([P, F], f32, tag="gst", bufs=3)
                    _emit_shifted_plane_load(nc, gst, g, k, s, zrow)
                    # |g| -> fp16 gate plane
                    nc.scalar.activation(wt[k][:], gst[:],
                                         mybir.ActivationFunctionType.Abs)
                    # wrap-column zeroing: Pool for the early planes (it
                    # idles all window), DVE for the last one
                    mt = maskLt if OFFSETS[k][1] == -1 else maskRt
                    eng = nc.vector if k == 7 else nc.gpsimd
                    eng.tensor_tensor(wt[k][:], wt[k][:], mt[:], MULT)
                    for q in range(8):
                        qs = q * CP
                        nc.tensor.matmul(psw[q][:], identt[:],
                                         wt[k][:, qs:qs + CP],
                                         start=(i == 0), stop=False)
                    if i == 3:
                        # blur mid-window: d0 / b ready long before the
                        # last gate plane lands
                        blur_st = prep.tile([P, F], f32, tag="gst", bufs=3)
                        nc.scalar.dma_start(
                            out=blur_st[:],
                            in_=blur[:].rearrange("(p f) -> p f", p=P))
                        # d0 = blur (fp16 body + halos); DVE so the ACT abs
                        # chain is never delayed
                        nc.vector.tensor_copy(out=dA[:, HALO:HALO + F],
                                              in_=blur_st[:])
                        _halo_dmas(nc, dA)
                        nc.vector.tensor_tensor(bt[:], m[:], blur_st[:], MULT)

                # dx=0 planes (no mask multiply) last, loaded column-chunked
                # and abs'd chunk-by-chunk: wsum bank 0 closes right after
                # the first chunks land instead of after the full planes
                gst1 = prep.tile([P, F], f32, tag="gst1", bufs=1)
                gst6 = prep.tile([P, F], f32, tag="gst6", bufs=1)
                s1, s6 = SHIFTS[1], SHIFTS[6]      # +1216 / -1216
                a6 = -s6
                nc.vector.memset(gst6[0:1, 0:a6], 0.0)
                lim1 = F - s1  # partition-127 in-bounds columns for k1
                for ci, (cs, cw) in enumerate(CHUNKS):
                    # k1 (s=+1216): partitions 0..126 shifted, 127 special
                    nc.sync.dma_start(
                        out=gst1[0:127, cs:cs + cw],
                        in_=g[1, s1:s1 + 127 * F].rearrange(
                            "(p f) -> p f", p=127)[:, cs:cs + cw])
                    if cs < lim1:
                        w = min(cs + cw, lim1) - cs
                        nc.sync.dma_start(
                            out=gst1[127:128, cs:cs + w],
                            in_=g[1, s1 + 127 * F + cs:s1 + 127 * F + cs + w]
                            .rearrange("(p f) -> p f", p=1))
                    if cs + cw > lim1:
                        zs = max(cs, lim1)
                        nc.sync.dma_start(out=gst1[127:128, zs:cs + cw],
                                          in_=zrow[0:1, 0:cs + cw - zs])
                    # k6 (s=-1216): partitions 1..127 shifted, 0 special
                    nc.scalar.dma_start(
                        out=gst6[1:128, cs:cs + cw],
                        in_=g[6, F - a6:F - a6 + 127 * F].rearrange(
                            "(p f) -> p f", p=127)[:, cs:cs + cw])
                    if cs + cw > a6:
                        zs = max(cs, a6)
                        nc.scalar.dma_start(
                            out=gst6[0:1, zs:cs + cw],
                            in_=g[6, zs - a6:cs + cw - a6].rearrange(
                                "(p f) -> p f", p=1))
                    for kk, gstk in ((1, gst1), (6, gst6)):
                        nc.scalar.activation(
                            wt[kk][:, cs:cs + cw], gstk[:, cs:cs + cw],
                            mybir.ActivationFunctionType.Abs)
                        for q in range(cs // CP, (cs + cw) // CP):
                            qs = q * CP
                            nc.tensor.matmul(psw[q][:], identt[:],
                                             wt[kk][:, qs:qs + CP],
                                             start=False, stop=(kk == 6))

                # winv = (1-mask)/wsum per chunk; the W_k normalize runs
                # fused into iteration 0 below
                winvh = prew
                for cs, cw in CHUNKS:
                    sl = slice(cs, cs + cw)
                    winv = prep.tile([P, CP], f32, tag="winv", bufs=2)
                    for q in range(cs // CP, (cs + cw) // CP):
                        qs = q * CP
                        nc.vector.reciprocal(winv[:], psw[q][:])
                        nc.vector.tensor_copy(out=winvh[:, qs:qs + CP],
                                              in_=winv[:])
                    nc.vector.tensor_tensor(winvh[:, sl], winvh[:, sl],
                                            mh[:, sl], MULT)

            # ---------------- 24 stencil iterations ----------------
            with tc.tile_pool(name="prod", bufs=3) as prodp, \
                 tc.tile_pool(name="psum", bufs=8, space="PSUM") as psump, \
                 tc.tile_pool(name="post", bufs=1) as postp:

                src, dst = dA, dB
                # chunks whose body feeds each halo side
                back_set = {c for c, (cs, cw) in enumerate(CHUNKS)
                            if cs < HALO}
                front_set = {c for c, (cs, cw) in enumerate(CHUNKS)
                             if cs + cw > F - HALO}
                ostage = postp.tile([P, F], f32)
                for it in range(PROP_TIME):
                    last = it == PROP_TIME - 1
                    order = range(NCHUNK) if it % 2 == 0 \
                        else range(NCHUNK - 1, -1, -1)
                    back_done = front_done = False
                    done = set()
                    prods = {}

                    def emit_pool(c):
                        cs, cw = CHUNKS[c]
                        if it == 0:
                            for k in (6, 7):
                                nc.gpsimd.tensor_tensor(
                                    wt[k][:, cs:cs + cw], wt[k][:, cs:cs + cw],
                                    prew[:, cs:cs + cw], MULT)
                        for k in (6, 7):
                            if (k, c) not in POOL_TASKS:
                                continue
                            base = HALO + SHIFTS[k]
                            pr = prodp.tile([P, 836], f16, tag=f"pr{k}",
                                            name=f"pr{k}")
                            nc.gpsimd.tensor_tensor(
                                pr[:, 0:cw], wt[k][:, cs:cs + cw],
                                src[:, base + cs:base + cs + cw], MULT)
                            prods[(k, c)] = pr

                    ordered = list(order)
                    # hoist the dependency-free b-stream (start) matmuls of
                    # all 8 PSUM banks to the iteration top: PE has warm-up
                    # work spanning the iteration-boundary bubble, so its
                    # p-state never drops to the cold 0.65 GHz tier
                    pstiles = {}
                    # reversed: the banks the previous iteration freed first
                    # come first, so this warm-up work is runnable immediately
                    for c in reversed(ordered):
                        cs, cw = CHUNKS[c]
                        for h in range(cw // CP):
                            hs = cs + h * CP
                            # stable per-bank PSUM tiles: bank X's warm-up
                            # matmul only waits on bank X's copy-out from the
                            # previous iteration
                            ps = psump.tile([P, CP], f32, name="ps",
                                            tag=f"ps{hs}", bufs=1)
                            nc.tensor.matmul(ps[:], identt[:],
                                             bt[:, hs:hs + CP],
                                             start=True, stop=False)
                            pstiles[hs] = ps
                    for idx, c in enumerate(ordered):
                        cs, cw = CHUNKS[c]
                        if idx == 0:
                            # Pool (GPSIMD) runs one chunk ahead: its tasks
                            # are ~3.3x slower than DVE's, so issuing them
                            # early keeps them off the PE bank critical path
                            emit_pool(c)
                        if it == 0:
                            # W_k = |g_k| * (1-mask)/wsum, fused chunk-wise
                            # into the first iteration so DVE never sits on
                            # a full normalize pass before iterating
                            for k in range(6):
                                nc.vector.tensor_tensor(
                                    wt[k][:, cs:cs + cw], wt[k][:, cs:cs + cw],
                                    prew[:, cs:cs + cw], MULT)
                        # vertical-shift planes (|s| >= 1215) read far from
                        # the chunk just written by the previous iteration;
                        # issue them first so the horizontal planes (s = +-1,
                        # which wait on the freshest PSUM->SBUF copies) don't
                        # stall the engine at iteration boundaries
                        dve_ks = [k for k in (0, 1, 2, 5, 6, 3, 4)
                                  if (k, c) not in POOL_TASKS]
                        for k in dve_ks:
                            base = HALO + SHIFTS[k]
                            pr = prodp.tile([P, 836], f16,
                                            tag=f"pr{k}", name=f"pr{k}")
                            nc.vector.tensor_tensor(
                                pr[:, 0:cw], wt[k][:, cs:cs + cw],
                                src[:, base + cs:base + cs + cw], MULT)
                            prods[(k, c)] = pr
                        if idx + 1 < len(ordered):
                            emit_pool(ordered[idx + 1])
                        for h in range(cw // CP):
                            hs = cs + h * CP
                            ps = pstiles[hs]
                            for k in range(8):
                                pk = prods[(k, c)]
                                nc.tensor.matmul(
                                    ps[:], identt[:],
                                    pk[:, hs - cs:hs - cs + CP],
                                    start=False, stop=(k == 7))
                            if last:
                                # stream final result straight to DRAM (fp32)
                                nc.scalar.copy(
                                    out=ostage[:, hs:hs + CP], in_=ps[:])
                                nc.sync.dma_start(
                                    out=out[:, hs:hs + CP],
                                    in_=ostage[:, hs:hs + CP])
                            else:
                                nc.scalar.copy(
                                    out=dst[:, HALO + hs:HALO + hs + CP],
                                    in_=ps[:])
                        if last:
                            continue
                        done.add(c)
                        # launch halo refreshes as soon as their source body
                        # chunks have been written
                        if not back_done and back_set <= done:
                            nc.sync.dma_start(
                                out=dst[0:127, HALO + F:HALO + F + HALO],
                                in_=dst[1:128, HALO:2 * HALO])
                            back_done = True
                        if not front_done and front_set <= done:
                            nc.scalar.dma_start(
                                out=dst[1:128, 0:HALO],
                                in_=dst[0:127, F:F + HALO])
                            front_done = True
                    src, dst = dst, src

    nc.finalize()
    _split_sync_waits(nc)
    return nc


def _consts():
    j = np.arange(HW, dtype=np.int64) % W
    mL = (j != 0).astype(np.float16).reshape(P, F)
    mR = (j != W - 1).astype(np.float16).reshape(P, F)
    return mL, mR, np.eye(P, dtype=np.float16)


def kernel(guidance, blur_depth, sparse_depth):
    if "nc" not in _CACHE:
        _CACHE["nc"] = _build()
    nc = _CACHE["nc"]
    guidance = np.asarray(guidance, dtype=np.float32)
    blur_depth = np.asarray(blur_depth, dtype=np.float32)
    sparse_depth = np.asarray(sparse_depth, dtype=np.float32)
    mL, mR, idm = _consts()
    in_maps = []
    for c in range(B):
        in_maps.append({
            "g": np.ascontiguousarray(guidance[c].reshape(8, HW)),
            "blur": np.ascontiguousarray(blur_depth[c].reshape(HW)),
            "sparse": np.ascontiguousarray(sparse_depth[c].reshape(HW)),
            "maskL": mL, "maskR": mR, "ident": idm,
        })
    # every iterate is a convex combination of blur_depth values, so the
    # output must stay inside blur's range; violations mean the device
    # glitched (transient NRT wedge) -> retry
    lo = float(blur_depth.min()) - 1e-2
    hi = float(blur_depth.max()) + 1e-2

    import time
    outp = None
    for attempt in range(4):
        try:
            res = run_bass_kernel_spmd(nc, in_maps, list(range(B)))
            outp = np.stack(
                [res.results[c]["out"].reshape(1, H, W) for c in range(B)])
            if np.isfinite(outp).all() and outp.min() >= lo and outp.max() <= hi:
                return outp
            print(f"kernel: attempt {attempt} produced out-of-range values; "
                  f"retrying", flush=True)
        except Exception as e:
            # transient NRT device-unrecoverable states clear on a retry
            if attempt == 3:
                raise
            print(f"kernel: attempt {attempt} failed ({type(e).__name__}); "
                  f"retrying", flush=True)
        time.sleep(20 * (attempt + 1))
    return outp
